# revision 3
# baseline (speedup 1.0000x reference)
"""Trainium2 Bass kernel for nn_BlockRevert.

Computation (per batch b, token s):
  out[b,s,0,:]   = temporal_block[b,s,0,:] + pe[s,:] + mod_emb[0,:]
  out[b,s,r+1,:] = (valid[b,s,idx] if idx<8 else mask_token) + pe[s,:] + mod_emb[r+1,:]
     where idx = revert_idx[b,s,r], valid[b,s,j] = temporal_block[b,s,1+j,:]

Sharding: data-parallel over batch, 1 batch per NeuronCore (8 cores).
Per core the gather is fully local.

The kernel is HBM-bandwidth-bound, so everything runs in bf16 (rel-err
budget is 2e-2; bf16 end-to-end costs ~5e-3): table, pe, mod and the
output are bf16, halving both the gather-read and store-write traffic
vs f32. The host converts inputs to bf16 and upcasts the output.

Table layout: 9 interleaved rows per token (8 valid + 1 mask-token
copy); the per-token mask copy spreads the ~2/3 mask-slot reads across
all HBM channels, and any revert index >= 8 points at that token's
mask row. The global slot (m=0) is read with one big sequential DMA
from a contiguous [S, D] tensor instead of per-token gather
descriptors.

Device program per core (token-major, 4 blocks of 128 tokens), index
and pe loads hoisted to the front:
  tile t[128 tokens, 17*512 bf16]:
    dma glb block      -> t[:, slot 0]
    dma_gather 4-slot chunks (dst[p, j, :] = tbl[idx[j*128+p], :])
    per chunk: += pe (broadcast over slots), += modrep, store
  Each chunk pipelines SDMA -> DVE -> SDMA independently.
"""

import os
import sys

import numpy as np

for _p in ("/opt/trn_rl_repo",):
    if _p not in sys.path and os.path.isdir(_p):
        sys.path.insert(0, _p)

import ml_dtypes

BF16 = ml_dtypes.bfloat16

B, S, MV, D, R = 8, 512, 8, 512, 16
NSLOT = R + 1          # 17 output slots
W = NSLOT * D          # 8704 elems per output row
NTR = MV + 1           # 9 table rows per token: 8 valid + mask copy
NT = S * NTR           # 4608 table rows per batch
BLK = 128              # tokens per block
NBLK = S // BLK
NIDX = BLK * R         # 2048 gathered rows per block (slots 1..16)
# gather chunk boundaries in slot space (slot 0 comes from the glb DMA);
# add/store chunks are (0,5,9,13,17)
GCHUNKS = (1, 5, 9, 13, 17)

MODE = os.environ.get("BLOCKREVERT_MODE", "bf16")


def _sinusoidal_pe(seq_len, d_model):
    pos = np.arange(seq_len)[:, None].astype(np.float32)
    div = np.exp(
        np.arange(0, d_model, 2).astype(np.float32) * (-np.log(10000.0) / d_model)
    )
    pe = np.zeros((seq_len, d_model), dtype=np.float32)
    pe[:, 0::2] = np.sin(pos * div)
    pe[:, 1::2] = np.cos(pos * div)
    return pe


def build_nc(mode=MODE, n_iter=None):
    import concourse.bacc as bacc
    import concourse.mybir as mybir
    import concourse.tile as tile

    bf16 = mybir.dt.bfloat16
    i16 = mybir.dt.int16

    nc = bacc.Bacc("TRN2", target_bir_lowering=False, debug=False)

    tbl = nc.dram_tensor("tbl", [NT, D], bf16, kind="ExternalInput")
    glb = nc.dram_tensor("glb", [S, D], bf16, kind="ExternalInput")
    # per-block dma_gather index buffers: wrapped into 16 partitions and
    # replicated across the 8 gpsimd cores -> [128, NIDX/16] per block
    gidx = nc.dram_tensor("gidx", [NBLK * BLK, NIDX // 16], i16, kind="ExternalInput")
    pe = nc.dram_tensor("pe", [S, D], bf16, kind="ExternalInput")
    modrep = nc.dram_tensor("modrep", [BLK, W], bf16, kind="ExternalInput")
    out = nc.dram_tensor("out", [S, W], bf16, kind="ExternalOutput")

    tbl_rows = tbl.ap()  # [NT, D]

    with tile.TileContext(nc) as tc:
        with (
            tc.tile_pool(name="const", bufs=1) as cpool,
            tc.tile_pool(name="work", bufs=3) as wpool,
            tc.tile_pool(name="small", bufs=3) as spool,
        ):

            def body():
                modt = cpool.tile([BLK, W], bf16)
                nc.sync.dma_start(out=modt[:], in_=modrep.ap())

                # hoist all index/pe loads so gathers start immediately
                its, pts = [], []
                for i in range(NBLK):
                    it = spool.tile([BLK, NIDX // 16], i16, tag=f"it{i}")
                    pt = spool.tile([BLK, D], bf16, tag=f"pt{i}")
                    nc.sync.dma_start(
                        out=it[:], in_=gidx.ap()[i * BLK : (i + 1) * BLK]
                    )
                    nc.sync.dma_start(out=pt[:], in_=pe.ap()[i * BLK : (i + 1) * BLK])
                    its.append(it)
                    pts.append(pt)

                for i in range(NBLK):
                    s0 = i * BLK
                    t = wpool.tile([BLK, W], bf16)
                    it, pt = its[i], pts[i]
                    # global slot: one sequential DMA into slot 0
                    nc.sync.dma_start(
                        out=t[:, 0:D], in_=glb.ap()[s0 : s0 + BLK]
                    )
                    # gathers for slots 1..16 (dst[p, j, :] = tbl[idx[j*128+p], :])
                    for ci in range(len(GCHUNKS) - 1):
                        slo, shi = GCHUNKS[ci], GCHUNKS[ci + 1]
                        nsl = shi - slo
                        per = nsl * BLK
                        tv = t[:, slo * D : shi * D].rearrange(
                            "p (m d) -> p m d", d=D
                        )
                        nc.gpsimd.dma_gather(
                            out_ap=tv,
                            in_ap=tbl_rows,
                            idxs_ap=it[
                                :, ((slo - 1) * BLK) // 16 : ((shi - 1) * BLK) // 16
                            ],
                            num_idxs=per,
                            num_idxs_reg=per,
                            elem_size=D,
                            single_packet=False,
                        )
                        # add/store chunk: include slot 0 in the first chunk
                        alo = 0 if ci == 0 else slo
                        ansl = shi - alo
                        av = t[:, alo * D : shi * D].rearrange(
                            "p (m d) -> p m d", d=D
                        )
                        pe_b = pt[:].unsqueeze(1).to_broadcast([BLK, ansl, D])
                        nc.vector.tensor_add(out=av, in0=av, in1=pe_b)
                        nc.vector.tensor_add(
                            out=t[:, alo * D : shi * D],
                            in0=t[:, alo * D : shi * D],
                            in1=modt[:, alo * D : shi * D],
                        )
                        nc.sync.dma_start(
                            out=out.ap()[s0 : s0 + BLK, alo * D : shi * D],
                            in_=t[:, alo * D : shi * D],
                        )

            if n_iter is None:
                body()
            else:
                with tc.For_i(0, n_iter):
                    body()

    nc.compile()
    return nc


def _wrap_idx(g_blk):
    """[BLK tokens, R slots] int16 -> dma_gather wrapped [BLK, NIDX/16]."""
    idxk = g_blk.T.reshape(-1)                # k = j*128 + p
    w16 = idxk.reshape(NIDX // 16, 16).T      # [16, NIDX/16]
    return np.tile(w16, (8, 1))               # replicate across gpsimd cores


def make_in_maps(temporal_block, mask_token, mod_emb, revert_idx, mode=MODE):
    temporal_block = np.asarray(temporal_block, dtype=np.float32)
    mask_token = np.asarray(mask_token, dtype=np.float32)
    mod_emb = np.asarray(mod_emb, dtype=np.float32)
    revert_idx = np.asarray(revert_idx)

    pe = _sinusoidal_pe(S, D).astype(BF16)
    modrep = np.ascontiguousarray(
        np.broadcast_to(mod_emb[:NSLOT].astype(BF16).reshape(1, W), (BLK, W))
    )

    # interleaved table: rows s*9+m = temporal_block[s,1+m] for m<8,
    # row s*9+8 = mask token (per-token copy -> HBM channel balance)
    tb16 = temporal_block.astype(BF16)
    mask16 = mask_token.astype(BF16)
    mask_col = np.broadcast_to(mask16, (B, S, 1, D))
    tbl_all = np.ascontiguousarray(
        np.concatenate([tb16[:, :, 1:, :], mask_col], axis=2)
    ).reshape(B, NT, D)
    glb_all = np.ascontiguousarray(tb16[:, :, 0, :])  # [B, S, D]

    # slot->table-row indices per token (slots 1..16): [B, S, R]
    idx_all = revert_idx.astype(np.int64)
    srow = np.arange(S, dtype=np.int64) * NTR  # [S]
    g_all = np.where(
        idx_all < MV, srow[None, :, None] + idx_all, srow[None, :, None] + MV
    ).astype(np.int16)

    in_maps = []
    for b in range(B):
        g = g_all[b]  # [S, R]
        gw = np.empty((NBLK, BLK, NIDX // 16), dtype=np.int16)
        for i in range(NBLK):
            gw[i] = _wrap_idx(g[i * BLK : (i + 1) * BLK])
        in_maps.append(
            {
                "tbl": tbl_all[b],
                "glb": glb_all[b],
                "gidx": np.ascontiguousarray(gw.reshape(NBLK * BLK, NIDX // 16)),
                "pe": pe,
                "modrep": modrep,
            }
        )
    return in_maps


def make_bench_arrays(rng, real_gidx=None):
    """Input arrays (one core's worth) for the bench repeat-loop."""
    gidx = real_gidx
    if gidx is None:
        g = rng.integers(0, NT, size=(NBLK, BLK, R), dtype=np.int16)
        gidx = np.ascontiguousarray(
            np.stack([_wrap_idx(g[i]) for i in range(NBLK)]).reshape(
                NBLK * BLK, NIDX // 16
            )
        )
    return {
        "tbl": rng.standard_normal((NT, D), dtype=np.float32).astype(BF16),
        "glb": rng.standard_normal((S, D), dtype=np.float32).astype(BF16),
        "gidx": gidx,
        "pe": rng.standard_normal((S, D), dtype=np.float32).astype(BF16),
        "modrep": rng.standard_normal((BLK, W), dtype=np.float32).astype(BF16),
    }


_CACHE = {}


def _get_nc(mode=MODE):
    if mode not in _CACHE:
        _CACHE[mode] = build_nc(mode)
    return _CACHE[mode]


def kernel(temporal_block, mask_token, mod_emb, revert_idx):
    from concourse.bass_utils import run_bass_kernel_spmd

    nc = _get_nc()
    in_maps = make_in_maps(temporal_block, mask_token, mod_emb, revert_idx)
    res = run_bass_kernel_spmd(nc, in_maps, core_ids=list(range(B)))
    out = np.stack(
        [
            np.asarray(res.results[b]["out"]).astype(np.float32).reshape(S, NSLOT, D)
            for b in range(B)
        ]
    )
    return out
